# revision 19
# baseline (speedup 1.0000x reference)
"""Trainium2 Bass kernel for nn_MeshNodeBlock (GNN message passing block).

reference semantics:
    agg = segment_sum(edge_features, src_indices, N)        # scatter-add
    x   = concat([node_features, agg], -1)
    h   = silu(x @ W1 + b1)
    y   = h @ W2 + b2
    y   = layer_norm(y) * gamma + beta
    out = y + node_features

Strategy (8 NeuronCores, SPMD, one NEFF):
  * Host assigns 128-node tiles to cores with a serpentine deal on per-tile
    chunk counts (load balance), sorts each core's tiles by count so the
    shared per-position chunk budget cis[i] = max over cores is tight.
  * Edges ship as bf16 feature rows plus a WIDTH-64 fp8 one-hot row (the
    128-node tile is split into two 64-node subtiles), 320B per edge slot;
    this keeps the DVE free of one-hot building at modest DMA cost.
  * Device works in transposed space (features on partitions, nodes free).
    Scatter-add per 128-node tile = ci PE matmuls aggT += edgeT @ onehot.
  * The layer-norm mean is folded into W2 on the host (W2c = W2 @ (I-J/128),
    b2c = b2 - mean(b2)), so layer 2 directly produces the mean-centered
    z0; variance is then just mean(z0^2) via one block-accumulated stats
    matmul per group, rstd = exp(-0.5 ln(var+eps)) once per block, and the
    per-node rstd row is partition-broadcast back via a tiny SWDGE DMA.
  * Normalize tail is 2 fused DVE ops per group:
    zb = (z0*gamma)*rstd_bc ; out = (zb+beta)+node, written bf16.
  * Emission is software-pipelined one group ahead (scatter of g+1 before
    MLP of g) so the PE never waits on the PSUM->SBUF agg copy or DMAs.
"""

import functools
from contextlib import ExitStack

import numpy as np
import ml_dtypes

import concourse.bass as bass
import concourse.tile as tile
from concourse import bacc, mybir
from concourse import bass_utils

BF16 = ml_dtypes.bfloat16
FP8 = ml_dtypes.float8_e4m3

N_NODES = 100000
D = 128
N_CORES = 8
P = 128
GROUP = 512              # nodes per group = 4 tiles
TILES_PER_CORE = 100
NODES_PER_CORE = TILES_PER_CORE * P   # 12800, 25 groups
EPS = 1e-5

AF = mybir.ActivationFunctionType
ALU = mybir.AluOpType
dt = mybir.dt


def _block_sizes(n_groups):
    """Phase blocks: fives, then a small tail block so the P3 drain is light."""
    if n_groups < 8:
        return [n_groups]
    sizes = []
    rem = n_groups
    while rem > 6:
        sizes.append(5)
        rem -= 5
    if rem > 1:
        sizes.append(rem - 1)
    sizes.append(1)
    return sizes


# --------------------------------------------------------------------------
# device kernel builder
# --------------------------------------------------------------------------

@functools.lru_cache(maxsize=4)
def _build(nodes_per_core: int, cis: tuple, n_cores: int):
    """cis: per tile position, (cA, cB) chunk budgets for the two subtiles."""
    assert nodes_per_core % GROUP == 0
    n_groups = nodes_per_core // GROUP
    tiles_per_core = nodes_per_core // P
    assert len(cis) == tiles_per_core

    # per-position byte layout inside pk: (cA+cB)*256 edge bytes + *64 onehot
    segb = [320 * (ca + cb) for ca, cb in cis]
    boff = np.concatenate([[0], np.cumsum(segb)]).astype(int)
    pk_bytes = int(boff[-1])
    gbytes_max = max(int(boff[g * 4 + 4] - boff[g * 4]) for g in range(n_groups))

    sizes = _block_sizes(n_groups)
    blocks = []
    g0b = 0
    for s in sizes:
        blocks.append(list(range(g0b, g0b + s)))
        g0b += s
    bmax = max(sizes)

    nc = bacc.Bacc("TRN2", target_bir_lowering=False, debug=False,
                   enable_asserts=False, num_devices=n_cores)

    PK = nc.dram_tensor("pk", [P, pk_bytes], dt.uint8, kind="ExternalInput").ap()
    NTB = nc.dram_tensor("ntb", [P, nodes_per_core], dt.bfloat16,
                         kind="ExternalInput").ap()
    W1P = nc.dram_tensor("w1p", [P, 1024], dt.bfloat16, kind="ExternalInput").ap()
    W2P = nc.dram_tensor("w2p", [P, 512], dt.bfloat16, kind="ExternalInput").ap()
    B1P = nc.dram_tensor("b1p", [P, 4], dt.float32, kind="ExternalInput").ap()
    B2P = nc.dram_tensor("b2p", [P, 1], dt.float32, kind="ExternalInput").ap()
    GAM = nc.dram_tensor("gam", [P, 1], dt.float32, kind="ExternalInput").ap()
    BET = nc.dram_tensor("bet", [P, 1], dt.float32, kind="ExternalInput").ap()
    ONB = nc.dram_tensor("onb", [P, bmax * 128], dt.bfloat16,
                         kind="ExternalInput").ap()
    OUT = nc.dram_tensor("out", [P, nodes_per_core], dt.bfloat16,
                         kind="ExternalOutput").ap()

    with tile.TileContext(nc) as tc:
        with ExitStack() as ctx:
            singles = ctx.enter_context(tc.tile_pool(name="singles", bufs=1))
            ebp = ctx.enter_context(tc.tile_pool(name="ebp", bufs=4))
            xtp = ctx.enter_context(tc.tile_pool(name="xtp", bufs=n_groups + 2))
            xap = ctx.enter_context(tc.tile_pool(name="xap", bufs=4))
            shp = ctx.enter_context(tc.tile_pool(name="shp", bufs=2))
            z0p = ctx.enter_context(tc.tile_pool(name="z0p", bufs=n_groups + 2))
            sqp = ctx.enter_context(tc.tile_pool(name="sqp", bufs=3))
            rsp = ctx.enter_context(tc.tile_pool(name="rsp", bufs=2))
            bcp = ctx.enter_context(tc.tile_pool(name="bcp", bufs=4))
            ofp = ctx.enter_context(tc.tile_pool(name="ofp", bufs=4))
            psagg = ctx.enter_context(tc.tile_pool(name="psagg", bufs=2, space="PSUM"))
            psh = ctx.enter_context(tc.tile_pool(name="psh", bufs=3, space="PSUM"))
            psz = ctx.enter_context(tc.tile_pool(name="psz", bufs=2, space="PSUM"))
            psst = ctx.enter_context(tc.tile_pool(name="psst", bufs=1, space="PSUM"))
            drp = ctx.enter_context(tc.tile_pool(name="drp", bufs=2, space="DRAM"))

            def load_const(name, src, shape, dtyp):
                t = singles.tile(shape, dtyp, tag=name)
                nc.scalar.dma_start(out=t[:], in_=src)
                return t

            w1 = load_const("w1", W1P, [P, 1024], dt.bfloat16)
            w2 = load_const("w2", W2P, [P, 512], dt.bfloat16)
            b1 = load_const("b1", B1P, [P, 4], dt.float32)
            b2c = load_const("b2c", B2P, [P, 1], dt.float32)
            gam = load_const("gam", GAM, [P, 1], dt.float32)
            bet = load_const("bet", BET, [P, 1], dt.float32)
            onb = load_const("onb", ONB, [P, bmax * 128], dt.bfloat16)
            eps = singles.tile([P, 1], dt.float32, tag="eps")
            nc.vector.memset(eps[:], EPS)

            xtn_tiles = {}
            z0_tiles = {}
            agg_tiles = {}
            _stats = {}

            def scat(g):
                """DMA + scatter matmuls for group g."""
                nsl = slice(g * GROUP, (g + 1) * GROUP)
                xtn = xtp.tile([P, GROUP], dt.bfloat16, tag="xtn")
                nc.sync.dma_start(out=xtn[:], in_=NTB[:, nsl])
                xtn_tiles[g] = xtn

                g0 = int(boff[g * 4])
                gbytes = int(boff[g * 4 + 4]) - g0
                pk = ebp.tile([P, gbytes_max], dt.uint8, tag="pk")
                nc.sync.dma_start(out=pk[:, :gbytes], in_=PK[:, g0:g0 + gbytes])

                agg_ps = psagg.tile([P, GROUP], dt.float32, tag="agg")
                for t4 in range(4):
                    ti = g * 4 + t4
                    ca, cb = cis[ti]
                    ct = ca + cb
                    toff = int(boff[ti]) - g0
                    ebv = pk[:, toff:toff + ct * 256].bitcast(dt.bfloat16)
                    ohv = pk[:, toff + ct * 256:toff + ct * 320].bitcast(
                        dt.float8e4)
                    for sub, (c0, cn) in enumerate(((0, ca), (ca, cb))):
                        for k in range(cn):
                            c = c0 + k
                            nc.tensor.matmul(
                                out=agg_ps[:, t4 * 128 + sub * 64:
                                           t4 * 128 + sub * 64 + 64],
                                lhsT=ebv[:, c * 128:(c + 1) * 128],
                                rhs=ohv[:, c * 64:(c + 1) * 64],
                                start=(k == 0), stop=(k == cn - 1))
                # PSUM -> SBUF bf16 copy
                xta = xap.tile([P, GROUP], dt.bfloat16, tag="xta")
                nc.vector.tensor_copy(out=xta[:], in_=agg_ps[:])
                agg_tiles[g] = xta

            sh_tiles = {}

            def l1(g):
                xtn = xtn_tiles[g]
                xta = agg_tiles.pop(g)
                tiles = []
                for j in range(4):
                    hps = psh.tile([P, GROUP], dt.float32, tag="hps")
                    nc.tensor.matmul(out=hps[:],
                                     lhsT=w1[:, j * 128:(j + 1) * 128],
                                     rhs=xtn[:], start=True, stop=False)
                    nc.tensor.matmul(
                        out=hps[:],
                        lhsT=w1[:, 512 + j * 128:512 + (j + 1) * 128],
                        rhs=xta[:], start=False, stop=True)
                    sh = shp.tile([P, GROUP], dt.bfloat16, tag=f"sh{j}")
                    nc.scalar.activation(out=sh[:], in_=hps[:], func=AF.Silu,
                                         bias=b1[:, j:j + 1], scale=1.0)
                    tiles.append(sh)
                sh_tiles[g] = tiles

            sq_tiles = {}

            def l2z(g):
                tiles = sh_tiles.pop(g)
                zps = psz.tile([P, GROUP], dt.float32, tag="zps")
                for j in range(4):
                    nc.tensor.matmul(out=zps[:],
                                     lhsT=w2[:, j * 128:(j + 1) * 128],
                                     rhs=tiles[j][:],
                                     start=(j == 0), stop=(j == 3))
                z0 = z0p.tile([P, GROUP], dt.bfloat16, tag="z0")
                nc.vector.tensor_scalar(out=z0[:], in0=zps[:],
                                        scalar1=b2c[:, 0:1], scalar2=None,
                                        op0=ALU.add)
                z0_tiles[g] = z0
                sq = sqp.tile([P, GROUP], dt.bfloat16, tag="sq")
                nc.vector.tensor_tensor(out=sq[:], in0=z0[:], in1=z0[:],
                                        op=ALU.mult)
                sq_tiles[g] = sq

            def stats(g, gi, bi, bsz):
                sq = sq_tiles.pop(g)
                if gi == 0:
                    _stats[bi] = psst.tile([P, GROUP], dt.float32, tag="m2",
                                           name=f"m2_{bi}")
                nc.tensor.matmul(out=_stats[bi][:],
                                 lhsT=onb[:, gi * 128:(gi + 1) * 128],
                                 rhs=sq[:],
                                 start=(gi == 0), stop=(gi == bsz - 1),
                                 skip_group_check=True)

            def phase2(bi, bsz):
                m2_ps = _stats.pop(bi)
                lnv = rsp.tile([P, GROUP], dt.bfloat16, tag="lnv")
                nc.scalar.activation(out=lnv[:], in_=m2_ps[:], func=AF.Ln,
                                     bias=eps[:, 0:1], scale=1.0)
                rstd = rsp.tile([P, GROUP], dt.bfloat16, tag="rstd")
                nc.scalar.activation(out=rstd[:], in_=lnv[:], func=AF.Exp,
                                     bias=0.0, scale=-0.5)
                bounce = drp.tile([bsz, GROUP], dt.bfloat16, tag="bounce")
                nc.gpsimd.dma_start(out=bounce[:], in_=rstd[0:bsz, :])
                return bounce

            def phase3(g, gi, bounce):
                nsl = slice(g * GROUP, (g + 1) * GROUP)
                bc = bcp.tile([P, GROUP], dt.bfloat16, tag="bc")
                bsl = bounce[gi:gi + 1, :]
                nc.gpsimd.dma_start(out=bc[:], in_=bass.AP(
                    tensor=bsl.tensor, offset=bsl.offset,
                    ap=[[0, P], bsl.ap[1]]))
                z0 = z0_tiles.pop(g)
                xtn = xtn_tiles.pop(g)
                zb = ofp.tile([P, GROUP], dt.bfloat16, tag="zb")
                nc.vector.tensor_tensor(out=zb[:], in0=z0[:], in1=bc[:],
                                        op=ALU.mult)
                zc = ofp.tile([P, GROUP], dt.bfloat16, tag="zc")
                nc.vector.tensor_scalar(out=zc[:], in0=zb[:],
                                        scalar1=gam[:, 0:1],
                                        scalar2=bet[:, 0:1],
                                        op0=ALU.mult, op1=ALU.add)
                of = ofp.tile([P, GROUP], dt.bfloat16, tag="of")
                nc.vector.tensor_tensor(out=of[:], in0=zc[:], in1=xtn[:],
                                        op=ALU.add)
                nc.gpsimd.dma_start(out=OUT[:, nsl], in_=of[:])

            # --- emission: 2-group scatter skew; PE order per iteration is
            # L1(g), scat(g+2), L2(g), stats(g-1) so no PE instruction waits
            # on same-iteration DVE/ACT work. P3 drains are paced evenly. ---
            g_of = []
            for bi, block in enumerate(blocks):
                for gi, g in enumerate(block):
                    g_of.append((g, gi, bi, len(block)))
            n = len(g_of)

            p3_queue = []
            stats_pending = None
            scat(g_of[0][0])
            if n > 1:
                scat(g_of[1][0])
            for idx, (g, gi, bi, bsz) in enumerate(g_of):
                l1(g)
                if idx + 2 < n:
                    scat(g_of[idx + 2][0])
                l2z(g)
                if stats_pending is not None:
                    pg, pgi, pbi, pbsz = stats_pending
                    stats(pg, pgi, pbi, pbsz)
                    if pgi == pbsz - 1:
                        bounce = phase2(pbi, pbsz)
                        p3_queue.extend((gg, ggi, bounce)
                                        for ggi, gg in enumerate(blocks[pbi]))
                stats_pending = (g, gi, bi, bsz)
                remaining = n - idx - 1
                k = (len(p3_queue) + max(remaining, 1) - 1) // max(remaining, 1)
                for _ in range(min(k, len(p3_queue))):
                    phase3(*p3_queue.pop(0))
            pg, pgi, pbi, pbsz = stats_pending
            stats(pg, pgi, pbi, pbsz)
            bounce = phase2(pbi, pbsz)
            p3_queue.extend((gg, ggi, bounce)
                            for ggi, gg in enumerate(blocks[pbi]))
            while p3_queue:
                phase3(*p3_queue.pop(0))

    nc.compile()
    return nc


# --------------------------------------------------------------------------
# host-side sharding / packing
# --------------------------------------------------------------------------

def _preprocess(inputs, n_cores, nodes_per_core):
    nf = np.ascontiguousarray(np.asarray(inputs["node_features"], np.float32))
    ef = np.ascontiguousarray(np.asarray(inputs["edge_features"], np.float32))
    src = np.asarray(inputs["src_indices"]).astype(np.int64)
    W1 = np.asarray(inputs["W1"], np.float32)
    b1 = np.asarray(inputs["b1"], np.float32)
    W2 = np.asarray(inputs["W2"], np.float32)
    b2 = np.asarray(inputs["b2"], np.float32)
    gam = np.asarray(inputs["ln_gamma"], np.float32)
    bet = np.asarray(inputs["ln_beta"], np.float32)

    n_nodes, d = nf.shape
    n_edges = ef.shape[0]
    tiles_per_core = nodes_per_core // P
    n_tiles = n_cores * tiles_per_core
    n_groups = nodes_per_core // GROUP
    bmax = max(_block_sizes(n_groups))

    # subtile = (tile, half); width-64 one-hot
    sub_e = src // 64                      # global subtile id, 2*n_tiles
    lid64 = (src % 64).astype(np.int64)
    scounts = np.bincount(sub_e, minlength=2 * n_tiles)
    scnt = np.ceil(scounts / P).astype(int).reshape(n_tiles, 2)

    # serpentine deal of tiles (desc by total chunk count) into cores, then
    # sort each core's tiles desc so the shared per-position budget is tight
    tot = scnt.sum(axis=1)
    order_t = np.argsort(-tot, kind="stable")
    assign = np.empty((n_cores, tiles_per_core), np.int64)
    for r in range(tiles_per_core):
        row = order_t[r * n_cores:(r + 1) * n_cores]
        if r % 2 == 1:
            row = row[::-1]
        assign[:, r] = row
    core_of_tile = np.empty(n_tiles, np.int64)
    pos_of_tile = np.empty(n_tiles, np.int64)
    for k in range(n_cores):
        core_of_tile[assign[k]] = k
        pos_of_tile[assign[k]] = np.arange(tiles_per_core)

    cisA = np.maximum(np.max(scnt[assign, 0], axis=0), 1).astype(int)
    cisB = np.maximum(np.max(scnt[assign, 1], axis=0), 1).astype(int)
    cist = cisA + cisB
    coff = np.concatenate([[0], np.cumsum(cist)]).astype(int)
    CH = int(coff[-1])

    # edge slot placement (within-subtile rank -> chunk, partition)
    order = np.argsort(src, kind="stable")
    snode = src[order]
    ssub = snode // 64
    sstarts = np.zeros(2 * n_tiles, np.int64)
    np.cumsum(scounts[:-1], out=sstarts[1:])
    rank = np.arange(n_edges, dtype=np.int64) - sstarts[ssub]
    chunk = rank // P
    part = rank % P
    stile = ssub // 2
    shalf = ssub % 2
    score = core_of_tile[stile]
    spos = pos_of_tile[stile]
    cslot = coff[spos] + np.where(shalf == 1, cisA[spos], 0) + chunk

    earr = np.zeros((n_cores, CH, P, d), BF16)
    earr[score, cslot, part] = ef[order].astype(BF16)
    oarr = np.zeros((n_cores, CH, P, 64), FP8)
    oarr[score, cslot, part, lid64[order]] = 1.0

    # pack pk bytes: per position, edge seg then one-hot seg
    segs = []
    for i in range(tiles_per_core):
        a, b = int(coff[i]), int(coff[i + 1])
        ci = b - a
        eseg = np.ascontiguousarray(
            earr[:, a:b].transpose(0, 2, 1, 3)).reshape(n_cores, P, ci * d)
        segs.append(eseg.view(np.uint8).reshape(n_cores, P, ci * 256))
        oseg = np.ascontiguousarray(
            oarr[:, a:b].transpose(0, 2, 1, 3)).reshape(n_cores, P, ci * 64)
        segs.append(oseg.view(np.uint8))
    PKa = np.ascontiguousarray(np.concatenate(segs, axis=2))

    # node features packed in assigned-tile order, transposed, bf16
    nfp = np.zeros((n_tiles * P, d), np.float32)
    nfp[:n_nodes] = nf
    tiles_nf = nfp.reshape(n_tiles, P, d)
    NTBa = np.empty((n_cores, P, nodes_per_core), BF16)
    for k in range(n_cores):
        blk = tiles_nf[assign[k]].reshape(nodes_per_core, d)
        NTBa[k] = blk.T.astype(BF16)

    # fold layernorm mean-centering into W2 / b2
    W2c = W2 - W2.mean(axis=1, keepdims=True)
    b2c = (b2 - b2.mean()).astype(np.float32)

    W1P = np.ascontiguousarray(
        W1.reshape(2, P, 4, P).transpose(1, 0, 2, 3).reshape(P, 1024)).astype(BF16)
    W2P = np.ascontiguousarray(
        W2c.reshape(4, P, P).transpose(1, 0, 2).reshape(P, 512)).astype(BF16)
    B1P = np.ascontiguousarray(b1.reshape(4, P).T)
    B2P = np.ascontiguousarray(b2c.reshape(P, 1))
    GAMP = np.ascontiguousarray(gam.reshape(P, 1))
    BETP = np.ascontiguousarray(bet.reshape(P, 1))
    ONB = np.zeros((P, bmax * 128), np.float32)
    for g in range(bmax):
        ONB[:, g * 128 + g] = 1.0 / P
    ONB = ONB.astype(BF16)

    in_maps = []
    for k in range(n_cores):
        in_maps.append({
            "pk": PKa[k], "ntb": NTBa[k],
            "w1p": W1P, "w2p": W2P, "b1p": B1P, "b2p": B2P,
            "gam": GAMP, "bet": BETP, "onb": ONB,
        })
    cis = tuple((int(a), int(b)) for a, b in zip(cisA, cisB))
    return in_maps, cis, assign


def _assemble(results, n_nodes, n_cores, nodes_per_core, assign):
    tiles_per_core = nodes_per_core // P
    n_tiles = n_cores * tiles_per_core
    full = np.empty((n_tiles, P, D), np.float32)
    for k in range(n_cores):
        outk = np.asarray(results[k]["out"]).astype(np.float32)  # [P, npc]
        full[assign[k]] = outk.T.reshape(tiles_per_core, P, D)
    return np.ascontiguousarray(full.reshape(n_tiles * P, D)[:n_nodes])


# --------------------------------------------------------------------------
# public entry point
# --------------------------------------------------------------------------

_AXON_SO = "/opt/axon/libaxon_pjrt.so"


def _ensure_ntff_hook():
    """Provide antenv.axon_hooks + register the ctypes NTFF profile hook
    (the agent image's antenv lacks axon_hooks, so boot degraded silently)."""
    import sys
    import types
    import ctypes
    import contextlib
    import os

    try:
        from antenv.axon_hooks import get_axon_ntff_profile_hook  # noqa: F401
        return
    except ImportError:
        pass
    import antenv

    m = types.ModuleType("antenv.axon_hooks")
    m._hook = None

    def set_axon_ntff_profile_hook(h):
        m._hook = h

    def get_axon_ntff_profile_hook():
        return m._hook

    m.set_axon_ntff_profile_hook = set_axon_ntff_profile_hook
    m.get_axon_ntff_profile_hook = get_axon_ntff_profile_hook
    sys.modules["antenv.axon_hooks"] = m
    antenv.axon_hooks = m

    if not os.path.exists(_AXON_SO):
        return
    lib = ctypes.CDLL(_AXON_SO)
    if not hasattr(lib, "axon_start_nrt_profile"):
        return
    lib.axon_start_nrt_profile.argtypes = [ctypes.POINTER(ctypes.c_int64),
                                           ctypes.c_size_t]
    lib.axon_start_nrt_profile.restype = ctypes.c_int64
    lib.axon_stop_nrt_profile.argtypes = [ctypes.c_char_p]
    lib.axon_stop_nrt_profile.restype = ctypes.c_int64

    @contextlib.contextmanager
    def _hook(output_dir, device_ids):
        import jax

        jax.devices()
        if device_ids:
            ids = (ctypes.c_int64 * len(device_ids))(*device_ids)
            rc = lib.axon_start_nrt_profile(ids, len(device_ids))
        else:
            rc = lib.axon_start_nrt_profile(None, 0)
        if rc != 0:
            raise RuntimeError(f"axon_start_nrt_profile rc={rc}")
        try:
            yield
        finally:
            n = lib.axon_stop_nrt_profile(str(output_dir).encode())
            if n < 0:
                raise RuntimeError(f"axon_stop_nrt_profile rc={n}")
            if n == 0:
                print("WARNING: NTFF capture wrote no files")

    m._hook = _hook


def _run(inputs, trace=False):
    if trace:
        _ensure_ntff_hook()
    n_nodes = np.asarray(inputs["node_features"]).shape[0]
    in_maps, cis, assign = _preprocess(inputs, N_CORES, NODES_PER_CORE)
    nc = _build(NODES_PER_CORE, cis, N_CORES)
    res = bass_utils.run_bass_kernel_spmd(
        nc, in_maps, core_ids=list(range(N_CORES)), trace=trace)
    out = _assemble(res.results, n_nodes, N_CORES, NODES_PER_CORE, assign)
    return out, res


def kernel(**inputs):
    out, _ = _run(inputs, trace=False)
    return out


def kernel_profiled(**inputs):
    out, res = _run(inputs, trace=True)
    return out, res


# revision 20
# speedup vs baseline: 1.1349x; 1.1349x over previous
"""Trainium2 Bass kernel for nn_MeshNodeBlock (GNN message passing block).

reference semantics:
    agg = segment_sum(edge_features, src_indices, N)        # scatter-add
    x   = concat([node_features, agg], -1)
    h   = silu(x @ W1 + b1)
    y   = h @ W2 + b2
    y   = layer_norm(y) * gamma + beta
    out = y + node_features

Strategy (8 NeuronCores, SPMD, one NEFF):
  * Host assigns 128-node tiles to cores with a serpentine deal on per-tile
    chunk counts (load balance), sorts each core's tiles by count so the
    shared per-position chunk budget cis[i] = max over cores is tight.
  * Edges ship as bf16 feature rows plus a WIDTH-64 fp8 one-hot row (the
    128-node tile is split into two 64-node subtiles), 320B per edge slot;
    this keeps the DVE free of one-hot building at modest DMA cost.
  * Device works in transposed space (features on partitions, nodes free).
    Scatter-add per 128-node tile = ci PE matmuls aggT += edgeT @ onehot.
  * The layer-norm mean is folded into W2 on the host (W2c = W2 @ (I-J/128),
    b2c = b2 - mean(b2)), so layer 2 directly produces the mean-centered
    z0; variance is then just mean(z0^2) via one block-accumulated stats
    matmul per group, rstd = exp(-0.5 ln(var+eps)) once per block, and the
    per-node rstd row is partition-broadcast back via a tiny SWDGE DMA.
  * Normalize tail is 2 fused DVE ops per group:
    zb = (z0*gamma)*rstd_bc ; out = (zb+beta)+node, written bf16.
  * Emission is software-pipelined one group ahead (scatter of g+1 before
    MLP of g) so the PE never waits on the PSUM->SBUF agg copy or DMAs.
"""

import functools
from contextlib import ExitStack

import numpy as np
import ml_dtypes

import concourse.bass as bass
import concourse.tile as tile
from concourse import bacc, mybir
from concourse import bass_utils

BF16 = ml_dtypes.bfloat16
FP8 = ml_dtypes.float8_e4m3

N_NODES = 100000
D = 128
N_CORES = 8
P = 128
GROUP = 512              # nodes per group = 4 tiles
TILES_PER_CORE = 100
NODES_PER_CORE = TILES_PER_CORE * P   # 12800, 25 groups
EPS = 1e-5

AF = mybir.ActivationFunctionType
ALU = mybir.AluOpType
dt = mybir.dt


def _block_sizes(n_groups):
    """Phase blocks: one big block then a small tail block. Few blocks keep
    the ACT table (Silu <-> Ln/Exp) from thrashing; the tail P3 stays light."""
    if n_groups < 8:
        return [n_groups]
    return [n_groups - 4, 4]


# --------------------------------------------------------------------------
# device kernel builder
# --------------------------------------------------------------------------

@functools.lru_cache(maxsize=4)
def _build(nodes_per_core: int, cis: tuple, n_cores: int):
    """cis: per tile position, (cA, cB) chunk budgets for the two subtiles."""
    assert nodes_per_core % GROUP == 0
    n_groups = nodes_per_core // GROUP
    tiles_per_core = nodes_per_core // P
    assert len(cis) == tiles_per_core

    # per-position byte layout inside pk: (cA+cB)*256 edge bytes + *64 onehot
    segb = [320 * (ca + cb) for ca, cb in cis]
    boff = np.concatenate([[0], np.cumsum(segb)]).astype(int)
    pk_bytes = int(boff[-1])
    gbytes_max = max(int(boff[g * 4 + 4] - boff[g * 4]) for g in range(n_groups))

    sizes = _block_sizes(n_groups)
    blocks = []
    g0b = 0
    for s in sizes:
        blocks.append(list(range(g0b, g0b + s)))
        g0b += s
    bmax = max(sizes)

    nc = bacc.Bacc("TRN2", target_bir_lowering=False, debug=False,
                   enable_asserts=False, num_devices=n_cores)

    PK = nc.dram_tensor("pk", [P, pk_bytes], dt.uint8, kind="ExternalInput").ap()
    NTB = nc.dram_tensor("ntb", [P, nodes_per_core], dt.bfloat16,
                         kind="ExternalInput").ap()
    W1P = nc.dram_tensor("w1p", [P, 1024], dt.bfloat16, kind="ExternalInput").ap()
    W2P = nc.dram_tensor("w2p", [P, 512], dt.bfloat16, kind="ExternalInput").ap()
    B1P = nc.dram_tensor("b1p", [P, 4], dt.float32, kind="ExternalInput").ap()
    B2P = nc.dram_tensor("b2p", [P, 1], dt.float32, kind="ExternalInput").ap()
    GAM = nc.dram_tensor("gam", [P, 1], dt.float32, kind="ExternalInput").ap()
    BET = nc.dram_tensor("bet", [P, 1], dt.float32, kind="ExternalInput").ap()
    ONB = nc.dram_tensor("onb", [P, bmax * 128], dt.bfloat16,
                         kind="ExternalInput").ap()
    OUT = nc.dram_tensor("out", [P, nodes_per_core], dt.bfloat16,
                         kind="ExternalOutput").ap()

    with tile.TileContext(nc) as tc:
        with ExitStack() as ctx:
            singles = ctx.enter_context(tc.tile_pool(name="singles", bufs=1))
            ebp = ctx.enter_context(tc.tile_pool(name="ebp", bufs=4))
            xtp = ctx.enter_context(tc.tile_pool(name="xtp", bufs=n_groups + 2))
            xap = ctx.enter_context(tc.tile_pool(name="xap", bufs=4))
            shp = ctx.enter_context(tc.tile_pool(name="shp", bufs=2))
            z0p = ctx.enter_context(tc.tile_pool(name="z0p", bufs=n_groups + 2))
            sqp = ctx.enter_context(tc.tile_pool(name="sqp", bufs=3))
            rsp = ctx.enter_context(tc.tile_pool(name="rsp", bufs=2))
            bcp = ctx.enter_context(tc.tile_pool(name="bcp", bufs=4))
            ofp = ctx.enter_context(tc.tile_pool(name="ofp", bufs=4))
            psagg = ctx.enter_context(tc.tile_pool(name="psagg", bufs=2, space="PSUM"))
            psh = ctx.enter_context(tc.tile_pool(name="psh", bufs=3, space="PSUM"))
            psz = ctx.enter_context(tc.tile_pool(name="psz", bufs=2, space="PSUM"))
            psst = ctx.enter_context(tc.tile_pool(name="psst", bufs=1, space="PSUM"))
            drp = ctx.enter_context(tc.tile_pool(name="drp", bufs=2, space="DRAM"))

            def load_const(name, src, shape, dtyp):
                t = singles.tile(shape, dtyp, tag=name)
                nc.scalar.dma_start(out=t[:], in_=src)
                return t

            w1 = load_const("w1", W1P, [P, 1024], dt.bfloat16)
            w2 = load_const("w2", W2P, [P, 512], dt.bfloat16)
            b1 = load_const("b1", B1P, [P, 4], dt.float32)
            b2c = load_const("b2c", B2P, [P, 1], dt.float32)
            gam = load_const("gam", GAM, [P, 1], dt.float32)
            bet = load_const("bet", BET, [P, 1], dt.float32)
            onb = load_const("onb", ONB, [P, bmax * 128], dt.bfloat16)
            eps = singles.tile([P, 1], dt.float32, tag="eps")
            nc.vector.memset(eps[:], EPS)

            xtn_tiles = {}
            z0_tiles = {}
            agg_tiles = {}
            _stats = {}

            def scat(g):
                """DMA + scatter matmuls for group g."""
                nsl = slice(g * GROUP, (g + 1) * GROUP)
                xtn = xtp.tile([P, GROUP], dt.bfloat16, tag="xtn")
                nc.sync.dma_start(out=xtn[:], in_=NTB[:, nsl])
                xtn_tiles[g] = xtn

                g0 = int(boff[g * 4])
                gbytes = int(boff[g * 4 + 4]) - g0
                pk = ebp.tile([P, gbytes_max], dt.uint8, tag="pk")
                nc.sync.dma_start(out=pk[:, :gbytes], in_=PK[:, g0:g0 + gbytes])

                agg_ps = psagg.tile([P, GROUP], dt.float32, tag="agg")
                for t4 in range(4):
                    ti = g * 4 + t4
                    ca, cb = cis[ti]
                    ct = ca + cb
                    toff = int(boff[ti]) - g0
                    ebv = pk[:, toff:toff + ct * 256].bitcast(dt.bfloat16)
                    ohv = pk[:, toff + ct * 256:toff + ct * 320].bitcast(
                        dt.float8e4)
                    for sub, (c0, cn) in enumerate(((0, ca), (ca, cb))):
                        for k in range(cn):
                            c = c0 + k
                            nc.tensor.matmul(
                                out=agg_ps[:, t4 * 128 + sub * 64:
                                           t4 * 128 + sub * 64 + 64],
                                lhsT=ebv[:, c * 128:(c + 1) * 128],
                                rhs=ohv[:, c * 64:(c + 1) * 64],
                                start=(k == 0), stop=(k == cn - 1))
                # PSUM -> SBUF bf16 copy
                xta = xap.tile([P, GROUP], dt.bfloat16, tag="xta")
                nc.vector.tensor_copy(out=xta[:], in_=agg_ps[:])
                agg_tiles[g] = xta

            sh_tiles = {}

            def l1(g):
                xtn = xtn_tiles[g]
                xta = agg_tiles.pop(g)
                tiles = []
                for j in range(4):
                    hps = psh.tile([P, GROUP], dt.float32, tag="hps")
                    nc.tensor.matmul(out=hps[:],
                                     lhsT=w1[:, j * 128:(j + 1) * 128],
                                     rhs=xtn[:], start=True, stop=False)
                    nc.tensor.matmul(
                        out=hps[:],
                        lhsT=w1[:, 512 + j * 128:512 + (j + 1) * 128],
                        rhs=xta[:], start=False, stop=True)
                    sh = shp.tile([P, GROUP], dt.bfloat16, tag=f"sh{j}")
                    nc.scalar.activation(out=sh[:], in_=hps[:], func=AF.Silu,
                                         bias=b1[:, j:j + 1], scale=1.0)
                    tiles.append(sh)
                sh_tiles[g] = tiles

            sq_tiles = {}

            def l2z(g):
                tiles = sh_tiles.pop(g)
                zps = psz.tile([P, GROUP], dt.float32, tag="zps")
                for j in range(4):
                    nc.tensor.matmul(out=zps[:],
                                     lhsT=w2[:, j * 128:(j + 1) * 128],
                                     rhs=tiles[j][:],
                                     start=(j == 0), stop=(j == 3))
                z0 = z0p.tile([P, GROUP], dt.bfloat16, tag="z0")
                nc.vector.tensor_scalar(out=z0[:], in0=zps[:],
                                        scalar1=b2c[:, 0:1], scalar2=None,
                                        op0=ALU.add)
                z0_tiles[g] = z0
                sq = sqp.tile([P, GROUP], dt.bfloat16, tag="sq")
                nc.vector.tensor_tensor(out=sq[:], in0=z0[:], in1=z0[:],
                                        op=ALU.mult)
                sq_tiles[g] = sq

            def stats(g, gi, bi, bsz):
                sq = sq_tiles.pop(g)
                if gi == 0:
                    _stats[bi] = psst.tile([P, GROUP], dt.float32, tag="m2",
                                           name=f"m2_{bi}")
                nc.tensor.matmul(out=_stats[bi][:],
                                 lhsT=onb[:, gi * 128:(gi + 1) * 128],
                                 rhs=sq[:],
                                 start=(gi == 0), stop=(gi == bsz - 1),
                                 skip_group_check=True)

            def phase2(bi, bsz):
                m2_ps = _stats.pop(bi)
                lnv = rsp.tile([P, GROUP], dt.bfloat16, tag="lnv")
                nc.scalar.activation(out=lnv[:], in_=m2_ps[:], func=AF.Ln,
                                     bias=eps[:, 0:1], scale=1.0)
                rstd = rsp.tile([P, GROUP], dt.bfloat16, tag="rstd")
                nc.scalar.activation(out=rstd[:], in_=lnv[:], func=AF.Exp,
                                     bias=0.0, scale=-0.5)
                bounce = drp.tile([bsz, GROUP], dt.bfloat16, tag="bounce")
                nc.gpsimd.dma_start(out=bounce[:], in_=rstd[0:bsz, :])
                return bounce

            def phase3(g, gi, bounce):
                nsl = slice(g * GROUP, (g + 1) * GROUP)
                bc = bcp.tile([P, GROUP], dt.bfloat16, tag="bc")
                bsl = bounce[gi:gi + 1, :]
                nc.gpsimd.dma_start(out=bc[:], in_=bass.AP(
                    tensor=bsl.tensor, offset=bsl.offset,
                    ap=[[0, P], bsl.ap[1]]))
                z0 = z0_tiles.pop(g)
                xtn = xtn_tiles.pop(g)
                zb = ofp.tile([P, GROUP], dt.bfloat16, tag="zb")
                nc.vector.tensor_tensor(out=zb[:], in0=z0[:], in1=bc[:],
                                        op=ALU.mult)
                zc = ofp.tile([P, GROUP], dt.bfloat16, tag="zc")
                nc.vector.tensor_scalar(out=zc[:], in0=zb[:],
                                        scalar1=gam[:, 0:1],
                                        scalar2=bet[:, 0:1],
                                        op0=ALU.mult, op1=ALU.add)
                of = ofp.tile([P, GROUP], dt.bfloat16, tag="of")
                nc.vector.tensor_tensor(out=of[:], in0=zc[:], in1=xtn[:],
                                        op=ALU.add)
                nc.gpsimd.dma_start(out=OUT[:, nsl], in_=of[:])

            # --- emission: 2-group scatter skew; PE order per iteration is
            # L1(g), scat(g+2), L2(g), stats(g-1) so no PE instruction waits
            # on same-iteration DVE/ACT work. P3 drains are paced evenly. ---
            g_of = []
            for bi, block in enumerate(blocks):
                for gi, g in enumerate(block):
                    g_of.append((g, gi, bi, len(block)))
            n = len(g_of)

            p3_queue = []
            stats_pending = None
            scat(g_of[0][0])
            if n > 1:
                scat(g_of[1][0])
            for idx, (g, gi, bi, bsz) in enumerate(g_of):
                l1(g)
                if idx + 2 < n:
                    scat(g_of[idx + 2][0])
                l2z(g)
                if stats_pending is not None:
                    pg, pgi, pbi, pbsz = stats_pending
                    stats(pg, pgi, pbi, pbsz)
                    if pgi == pbsz - 1:
                        bounce = phase2(pbi, pbsz)
                        p3_queue.extend((gg, ggi, bounce)
                                        for ggi, gg in enumerate(blocks[pbi]))
                stats_pending = (g, gi, bi, bsz)
                remaining = n - idx - 1
                k = (len(p3_queue) + max(remaining, 1) - 1) // max(remaining, 1)
                for _ in range(min(k, len(p3_queue))):
                    phase3(*p3_queue.pop(0))
            pg, pgi, pbi, pbsz = stats_pending
            stats(pg, pgi, pbi, pbsz)
            bounce = phase2(pbi, pbsz)
            p3_queue.extend((gg, ggi, bounce)
                            for ggi, gg in enumerate(blocks[pbi]))
            while p3_queue:
                phase3(*p3_queue.pop(0))

    nc.compile()
    return nc


# --------------------------------------------------------------------------
# host-side sharding / packing
# --------------------------------------------------------------------------

def _preprocess(inputs, n_cores, nodes_per_core):
    nf = np.ascontiguousarray(np.asarray(inputs["node_features"], np.float32))
    ef = np.ascontiguousarray(np.asarray(inputs["edge_features"], np.float32))
    src = np.asarray(inputs["src_indices"]).astype(np.int64)
    W1 = np.asarray(inputs["W1"], np.float32)
    b1 = np.asarray(inputs["b1"], np.float32)
    W2 = np.asarray(inputs["W2"], np.float32)
    b2 = np.asarray(inputs["b2"], np.float32)
    gam = np.asarray(inputs["ln_gamma"], np.float32)
    bet = np.asarray(inputs["ln_beta"], np.float32)

    n_nodes, d = nf.shape
    n_edges = ef.shape[0]
    tiles_per_core = nodes_per_core // P
    n_tiles = n_cores * tiles_per_core
    n_groups = nodes_per_core // GROUP
    bmax = max(_block_sizes(n_groups))

    # subtile = (tile, half); width-64 one-hot
    sub_e = src // 64                      # global subtile id, 2*n_tiles
    lid64 = (src % 64).astype(np.int64)
    scounts = np.bincount(sub_e, minlength=2 * n_tiles)
    scnt = np.ceil(scounts / P).astype(int).reshape(n_tiles, 2)

    # serpentine deal of tiles (desc by total chunk count) into cores, then
    # sort each core's tiles desc so the shared per-position budget is tight
    tot = scnt.sum(axis=1)
    order_t = np.argsort(-tot, kind="stable")
    assign = np.empty((n_cores, tiles_per_core), np.int64)
    for r in range(tiles_per_core):
        row = order_t[r * n_cores:(r + 1) * n_cores]
        if r % 2 == 1:
            row = row[::-1]
        assign[:, r] = row
    core_of_tile = np.empty(n_tiles, np.int64)
    pos_of_tile = np.empty(n_tiles, np.int64)
    for k in range(n_cores):
        core_of_tile[assign[k]] = k
        pos_of_tile[assign[k]] = np.arange(tiles_per_core)

    cisA = np.maximum(np.max(scnt[assign, 0], axis=0), 1).astype(int)
    cisB = np.maximum(np.max(scnt[assign, 1], axis=0), 1).astype(int)
    cist = cisA + cisB
    coff = np.concatenate([[0], np.cumsum(cist)]).astype(int)
    CH = int(coff[-1])

    # edge slot placement (within-subtile rank -> chunk, partition)
    order = np.argsort(src, kind="stable")
    snode = src[order]
    ssub = snode // 64
    sstarts = np.zeros(2 * n_tiles, np.int64)
    np.cumsum(scounts[:-1], out=sstarts[1:])
    rank = np.arange(n_edges, dtype=np.int64) - sstarts[ssub]
    chunk = rank // P
    part = rank % P
    stile = ssub // 2
    shalf = ssub % 2
    score = core_of_tile[stile]
    spos = pos_of_tile[stile]
    cslot = coff[spos] + np.where(shalf == 1, cisA[spos], 0) + chunk

    earr = np.zeros((n_cores, CH, P, d), BF16)
    earr[score, cslot, part] = ef[order].astype(BF16)
    oarr = np.zeros((n_cores, CH, P, 64), FP8)
    oarr[score, cslot, part, lid64[order]] = 1.0

    # pack pk bytes: per position, edge seg then one-hot seg
    segs = []
    for i in range(tiles_per_core):
        a, b = int(coff[i]), int(coff[i + 1])
        ci = b - a
        eseg = np.ascontiguousarray(
            earr[:, a:b].transpose(0, 2, 1, 3)).reshape(n_cores, P, ci * d)
        segs.append(eseg.view(np.uint8).reshape(n_cores, P, ci * 256))
        oseg = np.ascontiguousarray(
            oarr[:, a:b].transpose(0, 2, 1, 3)).reshape(n_cores, P, ci * 64)
        segs.append(oseg.view(np.uint8))
    PKa = np.ascontiguousarray(np.concatenate(segs, axis=2))

    # node features packed in assigned-tile order, transposed, bf16
    nfp = np.zeros((n_tiles * P, d), np.float32)
    nfp[:n_nodes] = nf
    tiles_nf = nfp.reshape(n_tiles, P, d)
    NTBa = np.empty((n_cores, P, nodes_per_core), BF16)
    for k in range(n_cores):
        blk = tiles_nf[assign[k]].reshape(nodes_per_core, d)
        NTBa[k] = blk.T.astype(BF16)

    # fold layernorm mean-centering into W2 / b2
    W2c = W2 - W2.mean(axis=1, keepdims=True)
    b2c = (b2 - b2.mean()).astype(np.float32)

    W1P = np.ascontiguousarray(
        W1.reshape(2, P, 4, P).transpose(1, 0, 2, 3).reshape(P, 1024)).astype(BF16)
    W2P = np.ascontiguousarray(
        W2c.reshape(4, P, P).transpose(1, 0, 2).reshape(P, 512)).astype(BF16)
    B1P = np.ascontiguousarray(b1.reshape(4, P).T)
    B2P = np.ascontiguousarray(b2c.reshape(P, 1))
    GAMP = np.ascontiguousarray(gam.reshape(P, 1))
    BETP = np.ascontiguousarray(bet.reshape(P, 1))
    ONB = np.zeros((P, bmax * 128), np.float32)
    for g in range(bmax):
        ONB[:, g * 128 + g] = 1.0 / P
    ONB = ONB.astype(BF16)

    in_maps = []
    for k in range(n_cores):
        in_maps.append({
            "pk": PKa[k], "ntb": NTBa[k],
            "w1p": W1P, "w2p": W2P, "b1p": B1P, "b2p": B2P,
            "gam": GAMP, "bet": BETP, "onb": ONB,
        })
    cis = tuple((int(a), int(b)) for a, b in zip(cisA, cisB))
    return in_maps, cis, assign


def _assemble(results, n_nodes, n_cores, nodes_per_core, assign):
    tiles_per_core = nodes_per_core // P
    n_tiles = n_cores * tiles_per_core
    full = np.empty((n_tiles, P, D), np.float32)
    for k in range(n_cores):
        outk = np.asarray(results[k]["out"]).astype(np.float32)  # [P, npc]
        full[assign[k]] = outk.T.reshape(tiles_per_core, P, D)
    return np.ascontiguousarray(full.reshape(n_tiles * P, D)[:n_nodes])


# --------------------------------------------------------------------------
# public entry point
# --------------------------------------------------------------------------

_AXON_SO = "/opt/axon/libaxon_pjrt.so"


def _ensure_ntff_hook():
    """Provide antenv.axon_hooks + register the ctypes NTFF profile hook
    (the agent image's antenv lacks axon_hooks, so boot degraded silently)."""
    import sys
    import types
    import ctypes
    import contextlib
    import os

    try:
        from antenv.axon_hooks import get_axon_ntff_profile_hook  # noqa: F401
        return
    except ImportError:
        pass
    import antenv

    m = types.ModuleType("antenv.axon_hooks")
    m._hook = None

    def set_axon_ntff_profile_hook(h):
        m._hook = h

    def get_axon_ntff_profile_hook():
        return m._hook

    m.set_axon_ntff_profile_hook = set_axon_ntff_profile_hook
    m.get_axon_ntff_profile_hook = get_axon_ntff_profile_hook
    sys.modules["antenv.axon_hooks"] = m
    antenv.axon_hooks = m

    if not os.path.exists(_AXON_SO):
        return
    lib = ctypes.CDLL(_AXON_SO)
    if not hasattr(lib, "axon_start_nrt_profile"):
        return
    lib.axon_start_nrt_profile.argtypes = [ctypes.POINTER(ctypes.c_int64),
                                           ctypes.c_size_t]
    lib.axon_start_nrt_profile.restype = ctypes.c_int64
    lib.axon_stop_nrt_profile.argtypes = [ctypes.c_char_p]
    lib.axon_stop_nrt_profile.restype = ctypes.c_int64

    @contextlib.contextmanager
    def _hook(output_dir, device_ids):
        import jax

        jax.devices()
        if device_ids:
            ids = (ctypes.c_int64 * len(device_ids))(*device_ids)
            rc = lib.axon_start_nrt_profile(ids, len(device_ids))
        else:
            rc = lib.axon_start_nrt_profile(None, 0)
        if rc != 0:
            raise RuntimeError(f"axon_start_nrt_profile rc={rc}")
        try:
            yield
        finally:
            n = lib.axon_stop_nrt_profile(str(output_dir).encode())
            if n < 0:
                raise RuntimeError(f"axon_stop_nrt_profile rc={n}")
            if n == 0:
                print("WARNING: NTFF capture wrote no files")

    m._hook = _hook


def _run(inputs, trace=False):
    if trace:
        _ensure_ntff_hook()
    n_nodes = np.asarray(inputs["node_features"]).shape[0]
    in_maps, cis, assign = _preprocess(inputs, N_CORES, NODES_PER_CORE)
    nc = _build(NODES_PER_CORE, cis, N_CORES)
    res = bass_utils.run_bass_kernel_spmd(
        nc, in_maps, core_ids=list(range(N_CORES)), trace=trace)
    out = _assemble(res.results, n_nodes, N_CORES, NODES_PER_CORE, assign)
    return out, res


def kernel(**inputs):
    out, _ = _run(inputs, trace=False)
    return out


def kernel_profiled(**inputs):
    out, res = _run(inputs, trace=True)
    return out, res


# revision 26
# speedup vs baseline: 1.2114x; 1.0674x over previous
"""Trainium2 Bass kernel for nn_MeshNodeBlock (GNN message passing block).

reference semantics:
    agg = segment_sum(edge_features, src_indices, N)        # scatter-add
    x   = concat([node_features, agg], -1)
    h   = silu(x @ W1 + b1)
    y   = h @ W2 + b2
    y   = layer_norm(y) * gamma + beta
    out = y + node_features

Strategy (8 NeuronCores, SPMD, one NEFF):
  * Host assigns 128-node tiles to cores with a serpentine deal on per-tile
    chunk counts (load balance), sorts each core's tiles by count so the
    shared per-position chunk budget cis[i] = max over cores is tight.
  * Edges ship as bf16 feature rows plus a WIDTH-64 fp8 one-hot row (the
    128-node tile is split into two 64-node subtiles), 320B per edge slot;
    this keeps the DVE free of one-hot building at modest DMA cost.
  * Device works in transposed space (features on partitions, nodes free).
    Scatter-add per 128-node tile = ci PE matmuls aggT += edgeT @ onehot.
  * The layer-norm mean is folded into W2 on the host (W2c = W2 @ (I-J/128),
    b2c = b2 - mean(b2)), so layer 2 directly produces the mean-centered
    z0; variance is then just mean(z0^2) via one block-accumulated stats
    matmul per group, rstd = exp(-0.5 ln(var+eps)) once per block, and the
    per-node rstd row is partition-broadcast back via a tiny SWDGE DMA.
  * Normalize tail is 2 fused DVE ops per group:
    zb = (z0*gamma)*rstd_bc ; out = (zb+beta)+node, written bf16.
  * Emission is software-pipelined one group ahead (scatter of g+1 before
    MLP of g) so the PE never waits on the PSUM->SBUF agg copy or DMAs.
"""

import functools
from contextlib import ExitStack

import numpy as np
import ml_dtypes

import concourse.bass as bass
import concourse.tile as tile
from concourse import bacc, mybir
from concourse import bass_utils

BF16 = ml_dtypes.bfloat16
FP8 = ml_dtypes.float8_e4m3

N_NODES = 100000
D = 128
N_CORES = 8
P = 128
GROUP = 512              # nodes per group = 4 tiles
TILES_PER_CORE = 100
NODES_PER_CORE = TILES_PER_CORE * P   # 12800, 25 groups
EPS = 1e-5

AF = mybir.ActivationFunctionType
ALU = mybir.AluOpType
dt = mybir.dt


def _block_sizes(n_groups):
    """Phase blocks: front-loaded big block (its P3 spreads over many later
    iterations), then shrinking blocks so the post-loop P3 tail is tiny.
    Each extra block costs ~2-4us of ACT table loads (Ln/Exp), so few."""
    if n_groups < 8:
        return [n_groups]
    sizes = [n_groups - 10, 5, 4, 1]
    return [s for s in sizes if s > 0]


# --------------------------------------------------------------------------
# device kernel builder
# --------------------------------------------------------------------------

@functools.lru_cache(maxsize=4)
def _build(nodes_per_core: int, cis: tuple, n_cores: int):
    """cis: per tile position, (cA, cB) chunk budgets for the two subtiles."""
    assert nodes_per_core % GROUP == 0
    n_groups = nodes_per_core // GROUP
    tiles_per_core = nodes_per_core // P
    assert len(cis) == tiles_per_core

    # per-position byte layout inside pk: (cA+cB)*256 edge bytes + *64 onehot
    segb = [320 * (ca + cb) for ca, cb in cis]
    boff = np.concatenate([[0], np.cumsum(segb)]).astype(int)
    pk_bytes = int(boff[-1])
    gbytes_max = max(int(boff[g * 4 + 4] - boff[g * 4]) for g in range(n_groups))

    sizes = _block_sizes(n_groups)
    blocks = []
    g0b = 0
    for s in sizes:
        blocks.append(list(range(g0b, g0b + s)))
        g0b += s
    bmax = max(sizes)

    nc = bacc.Bacc("TRN2", target_bir_lowering=False, debug=False,
                   enable_asserts=False, num_devices=n_cores)

    PK = nc.dram_tensor("pk", [P, pk_bytes], dt.uint8, kind="ExternalInput").ap()
    NTB = nc.dram_tensor("ntb", [P, nodes_per_core], dt.bfloat16,
                         kind="ExternalInput").ap()
    W1P = nc.dram_tensor("w1p", [P, 1024], dt.bfloat16, kind="ExternalInput").ap()
    W2P = nc.dram_tensor("w2p", [P, 512], dt.bfloat16, kind="ExternalInput").ap()
    B1P = nc.dram_tensor("b1p", [P, 4], dt.float32, kind="ExternalInput").ap()
    B2P = nc.dram_tensor("b2p", [P, 1], dt.float32, kind="ExternalInput").ap()
    GAM = nc.dram_tensor("gam", [P, 1], dt.float32, kind="ExternalInput").ap()
    BET = nc.dram_tensor("bet", [P, 1], dt.float32, kind="ExternalInput").ap()
    ONB = nc.dram_tensor("onb", [P, bmax * 128], dt.bfloat16,
                         kind="ExternalInput").ap()
    OUT = nc.dram_tensor("out", [P, nodes_per_core], dt.bfloat16,
                         kind="ExternalOutput").ap()

    with tile.TileContext(nc) as tc:
        with ExitStack() as ctx:
            singles = ctx.enter_context(tc.tile_pool(name="singles", bufs=1))
            ebp = ctx.enter_context(tc.tile_pool(name="ebp", bufs=4))
            xtp = ctx.enter_context(tc.tile_pool(name="xtp", bufs=n_groups + 2))
            xap = ctx.enter_context(tc.tile_pool(name="xap", bufs=4))
            shp = ctx.enter_context(tc.tile_pool(name="shp", bufs=2))
            z0p = ctx.enter_context(tc.tile_pool(name="z0p", bufs=n_groups + 2))
            sqp = ctx.enter_context(tc.tile_pool(name="sqp", bufs=3))
            rsp = ctx.enter_context(tc.tile_pool(name="rsp", bufs=2))
            bcp = ctx.enter_context(tc.tile_pool(name="bcp", bufs=2))
            ofp = ctx.enter_context(tc.tile_pool(name="ofp", bufs=4))
            psagg = ctx.enter_context(tc.tile_pool(name="psagg", bufs=2, space="PSUM"))
            psh = ctx.enter_context(tc.tile_pool(name="psh", bufs=3, space="PSUM"))
            psz = ctx.enter_context(tc.tile_pool(name="psz", bufs=2, space="PSUM"))
            psst = ctx.enter_context(tc.tile_pool(name="psst", bufs=1, space="PSUM"))
            drp = ctx.enter_context(tc.tile_pool(name="drp", bufs=2, space="DRAM"))

            def load_const(name, src, shape, dtyp):
                t = singles.tile(shape, dtyp, tag=name)
                nc.scalar.dma_start(out=t[:], in_=src)
                return t

            w1 = load_const("w1", W1P, [P, 1024], dt.bfloat16)
            w2 = load_const("w2", W2P, [P, 512], dt.bfloat16)
            b1 = load_const("b1", B1P, [P, 4], dt.float32)
            b2c = load_const("b2c", B2P, [P, 1], dt.float32)
            gam = load_const("gam", GAM, [P, 1], dt.float32)
            bet = load_const("bet", BET, [P, 1], dt.float32)
            onb = load_const("onb", ONB, [P, bmax * 128], dt.bfloat16)
            eps = singles.tile([P, 1], dt.float32, tag="eps")
            nc.vector.memset(eps[:], EPS)

            xtn_tiles = {}
            z0_tiles = {}
            agg_tiles = {}
            _stats = {}

            def scat(g):
                """DMA + scatter matmuls for group g."""
                nsl = slice(g * GROUP, (g + 1) * GROUP)
                xtn = xtp.tile([P, GROUP], dt.bfloat16, tag="xtn")
                nc.sync.dma_start(out=xtn[:], in_=NTB[:, nsl])
                xtn_tiles[g] = xtn

                g0 = int(boff[g * 4])
                gbytes = int(boff[g * 4 + 4]) - g0
                pk = ebp.tile([P, gbytes_max], dt.uint8, tag="pk")
                nc.sync.dma_start(out=pk[:, :gbytes], in_=PK[:, g0:g0 + gbytes])

                agg_ps = psagg.tile([P, GROUP], dt.float32, tag="agg")
                for t4 in range(4):
                    ti = g * 4 + t4
                    ca, cb = cis[ti]
                    ct = ca + cb
                    toff = int(boff[ti]) - g0
                    ebv = pk[:, toff:toff + ct * 256].bitcast(dt.bfloat16)
                    ohv = pk[:, toff + ct * 256:toff + ct * 320].bitcast(
                        dt.float8e4)
                    for sub, (c0, cn) in enumerate(((0, ca), (ca, cb))):
                        for k in range(cn):
                            c = c0 + k
                            nc.tensor.matmul(
                                out=agg_ps[:, t4 * 128 + sub * 64:
                                           t4 * 128 + sub * 64 + 64],
                                lhsT=ebv[:, c * 128:(c + 1) * 128],
                                rhs=ohv[:, c * 64:(c + 1) * 64],
                                start=(k == 0), stop=(k == cn - 1))
                # PSUM -> SBUF bf16 copy
                xta = xap.tile([P, GROUP], dt.bfloat16, tag="xta")
                nc.vector.tensor_copy(out=xta[:], in_=agg_ps[:])
                agg_tiles[g] = xta

            sh_tiles = {}

            def l1(g):
                xtn = xtn_tiles[g]
                xta = agg_tiles.pop(g)
                tiles = []
                for j in range(4):
                    hps = psh.tile([P, GROUP], dt.float32, tag="hps")
                    nc.tensor.matmul(out=hps[:],
                                     lhsT=w1[:, j * 128:(j + 1) * 128],
                                     rhs=xtn[:], start=True, stop=False)
                    nc.tensor.matmul(
                        out=hps[:],
                        lhsT=w1[:, 512 + j * 128:512 + (j + 1) * 128],
                        rhs=xta[:], start=False, stop=True)
                    sh = shp.tile([P, GROUP], dt.bfloat16, tag=f"sh{j}")
                    nc.scalar.activation(out=sh[:], in_=hps[:], func=AF.Silu,
                                         bias=b1[:, j:j + 1], scale=1.0)
                    tiles.append(sh)
                sh_tiles[g] = tiles

            sq_tiles = {}

            def l2z(g):
                tiles = sh_tiles.pop(g)
                zps = psz.tile([P, GROUP], dt.float32, tag="zps")
                for j in range(4):
                    nc.tensor.matmul(out=zps[:],
                                     lhsT=w2[:, j * 128:(j + 1) * 128],
                                     rhs=tiles[j][:],
                                     start=(j == 0), stop=(j == 3))
                z0 = z0p.tile([P, GROUP], dt.bfloat16, tag="z0")
                nc.vector.tensor_scalar(out=z0[:], in0=zps[:],
                                        scalar1=b2c[:, 0:1], scalar2=None,
                                        op0=ALU.add)
                z0_tiles[g] = z0
                sq = sqp.tile([P, GROUP], dt.bfloat16, tag="sq")
                nc.vector.tensor_tensor(out=sq[:], in0=z0[:], in1=z0[:],
                                        op=ALU.mult)
                sq_tiles[g] = sq

            def stats(g, gi, bi, bsz):
                sq = sq_tiles.pop(g)
                if gi == 0:
                    _stats[bi] = psst.tile([P, GROUP], dt.float32, tag="m2",
                                           name=f"m2_{bi}")
                nc.tensor.matmul(out=_stats[bi][:],
                                 lhsT=onb[:, gi * 128:(gi + 1) * 128],
                                 rhs=sq[:],
                                 start=(gi == 0), stop=(gi == bsz - 1),
                                 skip_group_check=True)

            def phase2(bi, bsz):
                m2_ps = _stats.pop(bi)
                lnv = rsp.tile([P, GROUP], dt.bfloat16, tag="lnv")
                nc.scalar.activation(out=lnv[:], in_=m2_ps[:], func=AF.Ln,
                                     bias=eps[:, 0:1], scale=1.0)
                rstd = rsp.tile([P, GROUP], dt.bfloat16, tag="rstd")
                nc.scalar.activation(out=rstd[:], in_=lnv[:], func=AF.Exp,
                                     bias=0.0, scale=-0.5)
                bounce = drp.tile([bsz, GROUP], dt.bfloat16, tag="bounce")
                nc.gpsimd.dma_start(out=bounce[:], in_=rstd[0:bsz, :])
                # one partition-broadcast DMA for the whole block: the DRAM
                # bounce tile is row-major, so read it as a single flat row
                # replicated across partitions (same 2D pattern as per-row).
                bc_big = bcp.tile([P, bmax * GROUP], dt.bfloat16, tag="bc")
                bsl = bounce[:]
                nc.gpsimd.dma_start(
                    out=bc_big[:, :bsz * GROUP],
                    in_=bass.AP(tensor=bsl.tensor, offset=bsl.offset,
                                ap=[[0, P], [1, bsz * GROUP]]))
                return bc_big

            def phase3(g, gi, bc_big):
                nsl = slice(g * GROUP, (g + 1) * GROUP)
                bc = bc_big[:, gi * GROUP:(gi + 1) * GROUP]
                z0 = z0_tiles.pop(g)
                xtn = xtn_tiles.pop(g)
                zb = ofp.tile([P, GROUP], dt.bfloat16, tag="zb")
                nc.vector.tensor_tensor(out=zb[:], in0=z0[:], in1=bc,
                                        op=ALU.mult)
                zc = ofp.tile([P, GROUP], dt.bfloat16, tag="zc")
                nc.vector.tensor_scalar(out=zc[:], in0=zb[:],
                                        scalar1=gam[:, 0:1],
                                        scalar2=bet[:, 0:1],
                                        op0=ALU.mult, op1=ALU.add)
                of = ofp.tile([P, GROUP], dt.bfloat16, tag="of")
                nc.vector.tensor_tensor(out=of[:], in0=zc[:], in1=xtn[:],
                                        op=ALU.add)
                nc.sync.dma_start(out=OUT[:, nsl], in_=of[:])

            # --- emission: 2-group scatter skew; PE order per iteration is
            # L1(g), scat(g+2), L2(g), stats(g-1) so no PE instruction waits
            # on same-iteration DVE/ACT work. P3 drains are paced evenly. ---
            g_of = []
            for bi, block in enumerate(blocks):
                for gi, g in enumerate(block):
                    g_of.append((g, gi, bi, len(block)))
            n = len(g_of)

            p3_queue = []
            stats_pending = None
            scat(g_of[0][0])
            if n > 1:
                scat(g_of[1][0])
            for idx, (g, gi, bi, bsz) in enumerate(g_of):
                l1(g)
                if idx + 2 < n:
                    scat(g_of[idx + 2][0])
                l2z(g)
                if stats_pending is not None:
                    pg, pgi, pbi, pbsz = stats_pending
                    stats(pg, pgi, pbi, pbsz)
                    if pgi == pbsz - 1:
                        bounce = phase2(pbi, pbsz)
                        p3_queue.extend((gg, ggi, bounce)
                                        for ggi, gg in enumerate(blocks[pbi]))
                stats_pending = (g, gi, bi, bsz)
                remaining = n - idx - 1
                k = (len(p3_queue) + max(remaining, 1) - 1) // max(remaining, 1)
                for _ in range(min(k, len(p3_queue))):
                    phase3(*p3_queue.pop(0))
            pg, pgi, pbi, pbsz = stats_pending
            stats(pg, pgi, pbi, pbsz)
            bounce = phase2(pbi, pbsz)
            p3_queue.extend((gg, ggi, bounce)
                            for ggi, gg in enumerate(blocks[pbi]))
            while p3_queue:
                phase3(*p3_queue.pop(0))

    nc.compile()
    return nc


# --------------------------------------------------------------------------
# host-side sharding / packing
# --------------------------------------------------------------------------

def _preprocess(inputs, n_cores, nodes_per_core):
    nf = np.ascontiguousarray(np.asarray(inputs["node_features"], np.float32))
    ef = np.ascontiguousarray(np.asarray(inputs["edge_features"], np.float32))
    src = np.asarray(inputs["src_indices"]).astype(np.int64)
    W1 = np.asarray(inputs["W1"], np.float32)
    b1 = np.asarray(inputs["b1"], np.float32)
    W2 = np.asarray(inputs["W2"], np.float32)
    b2 = np.asarray(inputs["b2"], np.float32)
    gam = np.asarray(inputs["ln_gamma"], np.float32)
    bet = np.asarray(inputs["ln_beta"], np.float32)

    n_nodes, d = nf.shape
    n_edges = ef.shape[0]
    tiles_per_core = nodes_per_core // P
    n_tiles = n_cores * tiles_per_core
    n_groups = nodes_per_core // GROUP
    bmax = max(_block_sizes(n_groups))

    # subtile = (tile, half); width-64 one-hot
    sub_e = src // 64                      # global subtile id, 2*n_tiles
    lid64 = (src % 64).astype(np.int64)
    scounts = np.bincount(sub_e, minlength=2 * n_tiles)
    scnt = np.ceil(scounts / P).astype(int).reshape(n_tiles, 2)

    # serpentine deal of tiles (desc by total chunk count) into cores, then
    # sort each core's tiles desc so the shared per-position budget is tight
    tot = scnt.sum(axis=1)
    order_t = np.argsort(-tot, kind="stable")
    assign = np.empty((n_cores, tiles_per_core), np.int64)
    for r in range(tiles_per_core):
        row = order_t[r * n_cores:(r + 1) * n_cores]
        if r % 2 == 1:
            row = row[::-1]
        assign[:, r] = row
    core_of_tile = np.empty(n_tiles, np.int64)
    pos_of_tile = np.empty(n_tiles, np.int64)
    for k in range(n_cores):
        core_of_tile[assign[k]] = k
        pos_of_tile[assign[k]] = np.arange(tiles_per_core)

    cisA = np.maximum(np.max(scnt[assign, 0], axis=0), 1).astype(int)
    cisB = np.maximum(np.max(scnt[assign, 1], axis=0), 1).astype(int)
    cist = cisA + cisB
    coff = np.concatenate([[0], np.cumsum(cist)]).astype(int)
    CH = int(coff[-1])

    # edge slot placement (within-subtile rank -> chunk, partition)
    order = np.argsort(src, kind="stable")
    snode = src[order]
    ssub = snode // 64
    sstarts = np.zeros(2 * n_tiles, np.int64)
    np.cumsum(scounts[:-1], out=sstarts[1:])
    rank = np.arange(n_edges, dtype=np.int64) - sstarts[ssub]
    chunk = rank // P
    part = rank % P
    stile = ssub // 2
    shalf = ssub % 2
    score = core_of_tile[stile]
    spos = pos_of_tile[stile]
    cslot = coff[spos] + np.where(shalf == 1, cisA[spos], 0) + chunk

    earr = np.zeros((n_cores, CH, P, d), BF16)
    earr[score, cslot, part] = ef[order].astype(BF16)
    oarr = np.zeros((n_cores, CH, P, 64), FP8)
    oarr[score, cslot, part, lid64[order]] = 1.0

    # pack pk bytes: per position, edge seg then one-hot seg
    segs = []
    for i in range(tiles_per_core):
        a, b = int(coff[i]), int(coff[i + 1])
        ci = b - a
        eseg = np.ascontiguousarray(
            earr[:, a:b].transpose(0, 2, 1, 3)).reshape(n_cores, P, ci * d)
        segs.append(eseg.view(np.uint8).reshape(n_cores, P, ci * 256))
        oseg = np.ascontiguousarray(
            oarr[:, a:b].transpose(0, 2, 1, 3)).reshape(n_cores, P, ci * 64)
        segs.append(oseg.view(np.uint8))
    PKa = np.ascontiguousarray(np.concatenate(segs, axis=2))

    # node features packed in assigned-tile order, transposed, bf16
    nfp = np.zeros((n_tiles * P, d), np.float32)
    nfp[:n_nodes] = nf
    tiles_nf = nfp.reshape(n_tiles, P, d)
    NTBa = np.empty((n_cores, P, nodes_per_core), BF16)
    for k in range(n_cores):
        blk = tiles_nf[assign[k]].reshape(nodes_per_core, d)
        NTBa[k] = blk.T.astype(BF16)

    # fold layernorm mean-centering into W2 / b2
    W2c = W2 - W2.mean(axis=1, keepdims=True)
    b2c = (b2 - b2.mean()).astype(np.float32)

    W1P = np.ascontiguousarray(
        W1.reshape(2, P, 4, P).transpose(1, 0, 2, 3).reshape(P, 1024)).astype(BF16)
    W2P = np.ascontiguousarray(
        W2c.reshape(4, P, P).transpose(1, 0, 2).reshape(P, 512)).astype(BF16)
    B1P = np.ascontiguousarray(b1.reshape(4, P).T)
    B2P = np.ascontiguousarray(b2c.reshape(P, 1))
    GAMP = np.ascontiguousarray(gam.reshape(P, 1))
    BETP = np.ascontiguousarray(bet.reshape(P, 1))
    ONB = np.zeros((P, bmax * 128), np.float32)
    for g in range(bmax):
        ONB[:, g * 128 + g] = 1.0 / P
    ONB = ONB.astype(BF16)

    in_maps = []
    for k in range(n_cores):
        in_maps.append({
            "pk": PKa[k], "ntb": NTBa[k],
            "w1p": W1P, "w2p": W2P, "b1p": B1P, "b2p": B2P,
            "gam": GAMP, "bet": BETP, "onb": ONB,
        })
    cis = tuple((int(a), int(b)) for a, b in zip(cisA, cisB))
    return in_maps, cis, assign


def _assemble(results, n_nodes, n_cores, nodes_per_core, assign):
    tiles_per_core = nodes_per_core // P
    n_tiles = n_cores * tiles_per_core
    full = np.empty((n_tiles, P, D), np.float32)
    for k in range(n_cores):
        outk = np.asarray(results[k]["out"]).astype(np.float32)  # [P, npc]
        full[assign[k]] = outk.T.reshape(tiles_per_core, P, D)
    return np.ascontiguousarray(full.reshape(n_tiles * P, D)[:n_nodes])


# --------------------------------------------------------------------------
# public entry point
# --------------------------------------------------------------------------

_AXON_SO = "/opt/axon/libaxon_pjrt.so"


def _ensure_ntff_hook():
    """Provide antenv.axon_hooks + register the ctypes NTFF profile hook
    (the agent image's antenv lacks axon_hooks, so boot degraded silently)."""
    import sys
    import types
    import ctypes
    import contextlib
    import os

    try:
        from antenv.axon_hooks import get_axon_ntff_profile_hook  # noqa: F401
        return
    except ImportError:
        pass
    import antenv

    m = types.ModuleType("antenv.axon_hooks")
    m._hook = None

    def set_axon_ntff_profile_hook(h):
        m._hook = h

    def get_axon_ntff_profile_hook():
        return m._hook

    m.set_axon_ntff_profile_hook = set_axon_ntff_profile_hook
    m.get_axon_ntff_profile_hook = get_axon_ntff_profile_hook
    sys.modules["antenv.axon_hooks"] = m
    antenv.axon_hooks = m

    if not os.path.exists(_AXON_SO):
        return
    lib = ctypes.CDLL(_AXON_SO)
    if not hasattr(lib, "axon_start_nrt_profile"):
        return
    lib.axon_start_nrt_profile.argtypes = [ctypes.POINTER(ctypes.c_int64),
                                           ctypes.c_size_t]
    lib.axon_start_nrt_profile.restype = ctypes.c_int64
    lib.axon_stop_nrt_profile.argtypes = [ctypes.c_char_p]
    lib.axon_stop_nrt_profile.restype = ctypes.c_int64

    @contextlib.contextmanager
    def _hook(output_dir, device_ids):
        import jax

        jax.devices()
        if device_ids:
            ids = (ctypes.c_int64 * len(device_ids))(*device_ids)
            rc = lib.axon_start_nrt_profile(ids, len(device_ids))
        else:
            rc = lib.axon_start_nrt_profile(None, 0)
        if rc != 0:
            raise RuntimeError(f"axon_start_nrt_profile rc={rc}")
        try:
            yield
        finally:
            n = lib.axon_stop_nrt_profile(str(output_dir).encode())
            if n < 0:
                raise RuntimeError(f"axon_stop_nrt_profile rc={n}")
            if n == 0:
                print("WARNING: NTFF capture wrote no files")

    m._hook = _hook


def _run(inputs, trace=False):
    if trace:
        _ensure_ntff_hook()
    n_nodes = np.asarray(inputs["node_features"]).shape[0]
    in_maps, cis, assign = _preprocess(inputs, N_CORES, NODES_PER_CORE)
    nc = _build(NODES_PER_CORE, cis, N_CORES)
    res = bass_utils.run_bass_kernel_spmd(
        nc, in_maps, core_ids=list(range(N_CORES)), trace=trace)
    out = _assemble(res.results, n_nodes, N_CORES, NODES_PER_CORE, assign)
    return out, res


def kernel(**inputs):
    out, _ = _run(inputs, trace=False)
    return out


def kernel_profiled(**inputs):
    out, res = _run(inputs, trace=True)
    return out, res


# revision 37
# speedup vs baseline: 1.3002x; 1.0733x over previous
"""Trainium2 Bass kernel for nn_MeshNodeBlock (GNN message passing block).

reference semantics:
    agg = segment_sum(edge_features, src_indices, N)        # scatter-add
    x   = concat([node_features, agg], -1)
    h   = silu(x @ W1 + b1)
    y   = h @ W2 + b2
    y   = layer_norm(y) * gamma + beta
    out = y + node_features

Strategy (8 NeuronCores, SPMD, one NEFF):
  * Host assigns 128-node tiles to cores with a serpentine deal on per-tile
    chunk counts (load balance), sorts each core's tiles by count so the
    shared per-position chunk budget cis[i] = max over cores is tight.
  * Edges ship as bf16 feature rows plus a WIDTH-64 fp8 one-hot row (the
    128-node tile is split into two 64-node subtiles), 320B per edge slot;
    this keeps the DVE free of one-hot building at modest DMA cost.
  * Device works in transposed space (features on partitions, nodes free).
    Scatter-add per 128-node tile = ci PE matmuls aggT += edgeT @ onehot.
  * The layer-norm mean is folded into W2 on the host (W2c = W2 @ (I-J/128),
    b2c = b2 - mean(b2)), so layer 2 directly produces the mean-centered
    z0; variance is then just mean(z0^2) via one block-accumulated stats
    matmul per group, rstd = exp(-0.5 ln(var+eps)) once per block, and the
    per-node rstd row is partition-broadcast back via a tiny SWDGE DMA.
  * Normalize tail is 2 fused DVE ops per group:
    zb = (z0*gamma)*rstd_bc ; out = (zb+beta)+node, written bf16.
  * Emission is software-pipelined one group ahead (scatter of g+1 before
    MLP of g) so the PE never waits on the PSUM->SBUF agg copy or DMAs.
"""

import functools
from contextlib import ExitStack

import numpy as np
import ml_dtypes

import concourse.bass as bass
import concourse.tile as tile
from concourse import bacc, mybir
from concourse import bass_utils

BF16 = ml_dtypes.bfloat16
FP8 = ml_dtypes.float8_e4m3

N_NODES = 100000
D = 128
N_CORES = 8
P = 128
GROUP = 512              # nodes per group = 4 tiles
TILES_PER_CORE = 100
NODES_PER_CORE = TILES_PER_CORE * P   # 12800, 25 groups
EPS = 1e-5

AF = mybir.ActivationFunctionType
ALU = mybir.AluOpType
dt = mybir.dt


def _block_sizes(n_groups):
    """Phase blocks: front-loaded big block (its P3 spreads over many later
    iterations), then shrinking blocks so the post-loop P3 tail is tiny.
    Each extra block costs ~2-4us of ACT table loads (Ln/Exp), so few."""
    if n_groups < 8:
        return [n_groups]
    sizes = [n_groups - 10, 5, 4, 1]
    return [s for s in sizes if s > 0]


# --------------------------------------------------------------------------
# device kernel builder
# --------------------------------------------------------------------------

@functools.lru_cache(maxsize=4)
def _build(nodes_per_core: int, cis: tuple, n_cores: int):
    """cis: per tile position, (cA, cB) chunk budgets for the two subtiles."""
    assert nodes_per_core % GROUP == 0
    n_groups = nodes_per_core // GROUP
    tiles_per_core = nodes_per_core // P
    assert len(cis) == tiles_per_core

    # per-position byte layout inside pk: (cA+cB)*256 edge bytes + *64 onehot
    segb = [320 * (ca + cb) for ca, cb in cis]
    boff = np.concatenate([[0], np.cumsum(segb)]).astype(int)
    pk_bytes = int(boff[-1])
    gbytes_max = max(int(boff[g * 4 + 4] - boff[g * 4]) for g in range(n_groups))

    sizes = _block_sizes(n_groups)
    blocks = []
    g0b = 0
    for s in sizes:
        blocks.append(list(range(g0b, g0b + s)))
        g0b += s
    bmax = max(sizes)

    nc = bacc.Bacc("TRN2", target_bir_lowering=False, debug=False,
                   enable_asserts=False, num_devices=n_cores)

    PK = nc.dram_tensor("pk", [P, pk_bytes], dt.uint8, kind="ExternalInput").ap()
    NTB = nc.dram_tensor("ntb", [P, nodes_per_core], dt.bfloat16,
                         kind="ExternalInput").ap()
    W1P = nc.dram_tensor("w1p", [P, 1024], dt.bfloat16, kind="ExternalInput").ap()
    W2P = nc.dram_tensor("w2p", [P, 512], dt.bfloat16, kind="ExternalInput").ap()
    B1P = nc.dram_tensor("b1p", [P, 4], dt.float32, kind="ExternalInput").ap()
    B2P = nc.dram_tensor("b2p", [P, 1], dt.float32, kind="ExternalInput").ap()
    GAM = nc.dram_tensor("gam", [P, 1], dt.float32, kind="ExternalInput").ap()
    BET = nc.dram_tensor("bet", [P, 1], dt.float32, kind="ExternalInput").ap()
    ONB = nc.dram_tensor("onb", [P, bmax * 128], dt.bfloat16,
                         kind="ExternalInput").ap()
    bmax_late = max(sizes[1:], default=0)
    SEL = nc.dram_tensor("sel", [P, max(bmax_late, 1) * 128], dt.bfloat16,
                         kind="ExternalInput").ap()
    OUT = nc.dram_tensor("out", [P, nodes_per_core], dt.bfloat16,
                         kind="ExternalOutput").ap()

    with tile.TileContext(nc) as tc:
        with ExitStack() as ctx:
            singles = ctx.enter_context(tc.tile_pool(name="singles", bufs=1))
            ebp = ctx.enter_context(tc.tile_pool(name="ebp", bufs=5))
            xtp = ctx.enter_context(tc.tile_pool(name="xtp", bufs=n_groups + 2))
            xap = ctx.enter_context(tc.tile_pool(name="xap", bufs=5))
            shp = ctx.enter_context(tc.tile_pool(name="shp", bufs=2))
            z0p = ctx.enter_context(tc.tile_pool(name="z0p", bufs=n_groups + 2))
            sqp = ctx.enter_context(tc.tile_pool(name="sqp", bufs=3))
            rsp = ctx.enter_context(tc.tile_pool(name="rsp", bufs=3))
            bcp = ctx.enter_context(tc.tile_pool(name="bcp", bufs=2))
            ofp = ctx.enter_context(tc.tile_pool(name="ofp", bufs=4))
            psagg = ctx.enter_context(tc.tile_pool(name="psagg", bufs=2, space="PSUM"))
            psh = ctx.enter_context(tc.tile_pool(name="psh", bufs=3, space="PSUM"))
            psz = ctx.enter_context(tc.tile_pool(name="psz", bufs=1, space="PSUM"))
            psbc = ctx.enter_context(tc.tile_pool(name="psbc", bufs=1, space="PSUM"))
            psst = ctx.enter_context(tc.tile_pool(name="psst", bufs=1, space="PSUM"))
            drp = ctx.enter_context(tc.tile_pool(name="drp", bufs=2, space="DRAM"))

            def load_const(name, src, shape, dtyp):
                t = singles.tile(shape, dtyp, tag=name)
                nc.scalar.dma_start(out=t[:], in_=src)
                return t

            w1 = load_const("w1", W1P, [P, 1024], dt.bfloat16)
            w2 = load_const("w2", W2P, [P, 512], dt.bfloat16)
            b1 = load_const("b1", B1P, [P, 4], dt.float32)
            b2c = load_const("b2c", B2P, [P, 1], dt.float32)
            gam = load_const("gam", GAM, [P, 1], dt.float32)
            bet = load_const("bet", BET, [P, 1], dt.float32)
            onb = load_const("onb", ONB, [P, bmax * 128], dt.bfloat16)
            sel = load_const("sel", SEL, [P, max(bmax_late, 1) * 128],
                             dt.bfloat16)
            eps = singles.tile([P, 1], dt.float32, tag="eps")
            nc.vector.memset(eps[:], EPS)

            xtn_tiles = {}
            z0_tiles = {}
            agg_tiles = {}
            _stats = {}

            def scat(g):
                """DMA + scatter matmuls for group g."""
                nsl = slice(g * GROUP, (g + 1) * GROUP)
                g0 = int(boff[g * 4])
                gbytes = int(boff[g * 4 + 4]) - g0
                pk = ebp.tile([P, gbytes_max], dt.uint8, tag="pk")
                nc.sync.dma_start(out=pk[:, :gbytes], in_=PK[:, g0:g0 + gbytes])
                xtn = xtp.tile([P, GROUP], dt.bfloat16, tag="xtn")
                nc.sync.dma_start(out=xtn[:], in_=NTB[:, nsl])
                xtn_tiles[g] = xtn

                agg_ps = psagg.tile([P, GROUP], dt.float32, tag="agg")
                for t4 in range(4):
                    ti = g * 4 + t4
                    ca, cb = cis[ti]
                    ct = ca + cb
                    toff = int(boff[ti]) - g0
                    ebv = pk[:, toff:toff + ct * 256].bitcast(dt.bfloat16)
                    ohv = pk[:, toff + ct * 256:toff + ct * 320].bitcast(
                        dt.float8e4)
                    for sub, (c0, cn) in enumerate(((0, ca), (ca, cb))):
                        for k in range(cn):
                            c = c0 + k
                            nc.tensor.matmul(
                                out=agg_ps[:, t4 * 128 + sub * 64:
                                           t4 * 128 + sub * 64 + 64],
                                lhsT=ebv[:, c * 128:(c + 1) * 128],
                                rhs=ohv[:, c * 64:(c + 1) * 64],
                                start=(k == 0), stop=(k == cn - 1))
                # PSUM -> SBUF bf16 copy
                xta = xap.tile([P, GROUP], dt.bfloat16, tag="xta")
                nc.vector.tensor_copy(out=xta[:], in_=agg_ps[:])
                agg_tiles[g] = xta

            sh_tiles = {}
            xta_live = {}

            def l1j(g, j):
                xtn = xtn_tiles[g]
                xta = xta_live[g]
                hps = psh.tile([P, GROUP], dt.float32, tag="hps")
                nc.tensor.matmul(out=hps[:],
                                 lhsT=w1[:, j * 128:(j + 1) * 128],
                                 rhs=xtn[:], start=True, stop=False)
                nc.tensor.matmul(
                    out=hps[:],
                    lhsT=w1[:, 512 + j * 128:512 + (j + 1) * 128],
                    rhs=xta[:], start=False, stop=True)
                sh = shp.tile([P, GROUP], dt.bfloat16, tag=f"sh{j}")
                nc.scalar.activation(out=sh[:], in_=hps[:], func=AF.Silu,
                                     bias=b1[:, j:j + 1], scale=1.0)
                sh_tiles.setdefault(g, []).append(sh)

            def l1a(g):
                xta_live[g] = agg_tiles.pop(g)
                for j in range(3):
                    l1j(g, j)

            def l1b(g):
                l1j(g, 3)
                del xta_live[g]

            sq_tiles = {}

            def l2z(g):
                tiles = sh_tiles.pop(g)
                zps = psz.tile([P, GROUP], dt.float32, tag="zps")
                for j in range(4):
                    nc.tensor.matmul(out=zps[:],
                                     lhsT=w2[:, j * 128:(j + 1) * 128],
                                     rhs=tiles[j][:],
                                     start=(j == 0), stop=(j == 3))
                z0 = z0p.tile([P, GROUP], dt.bfloat16, tag="z0")
                nc.vector.tensor_scalar(out=z0[:], in0=zps[:],
                                        scalar1=b2c[:, 0:1], scalar2=None,
                                        op0=ALU.add)
                z0_tiles[g] = z0
                sq = sqp.tile([P, GROUP], dt.bfloat16, tag="sq")
                nc.vector.tensor_tensor(out=sq[:], in0=z0[:], in1=z0[:],
                                        op=ALU.mult)
                sq_tiles[g] = sq

            def stats(g, gi, bi, bsz):
                sq = sq_tiles.pop(g)
                if gi == 0:
                    _stats[bi] = psst.tile([P, GROUP], dt.float32, tag="m2",
                                           name=f"m2_{bi}")
                nc.tensor.matmul(out=_stats[bi][:],
                                 lhsT=onb[:, gi * 128:(gi + 1) * 128],
                                 rhs=sq[:],
                                 start=(gi == 0), stop=(gi == bsz - 1),
                                 skip_group_check=True)

            def phase2(bi, bsz):
                m2_ps = _stats.pop(bi)
                lnv = rsp.tile([P, GROUP], dt.bfloat16, tag="lnv")
                nc.scalar.activation(out=lnv[:], in_=m2_ps[:], func=AF.Ln,
                                     bias=eps[:, 0:1], scale=1.0)
                rstd = rsp.tile([P, GROUP], dt.bfloat16, tag="rstd")
                nc.scalar.activation(out=rstd[:], in_=lnv[:], func=AF.Exp,
                                     bias=0.0, scale=-0.5)
                if bi == 0:
                    # one partition-broadcast DMA for the whole block (DRAM
                    # bounce is row-major -> flat replicated-row read)
                    bounce = drp.tile([bsz, GROUP], dt.bfloat16, tag="bounce")
                    nc.gpsimd.dma_start(out=bounce[:], in_=rstd[0:bsz, :])
                    bc_big = bcp.tile([P, bmax * GROUP], dt.bfloat16, tag="bc")
                    bsl = bounce[:]
                    nc.gpsimd.dma_start(
                        out=bc_big[:, :bsz * GROUP],
                        in_=bass.AP(tensor=bsl.tensor, offset=bsl.offset,
                                    ap=[[0, P], [1, bsz * GROUP]]))
                    return ("sbuf", bc_big)
                # late blocks: row-select broadcast on the (now idle) PE
                return ("psum", rstd)

            def phase3(g, gi, bcinfo):
                nsl = slice(g * GROUP, (g + 1) * GROUP)
                mode, src = bcinfo
                if mode == "sbuf":
                    bc = src[:, gi * GROUP:(gi + 1) * GROUP]
                else:
                    bc_ps = psbc.tile([P, GROUP], dt.float32, tag="bcps")
                    nc.tensor.matmul(out=bc_ps[:],
                                     lhsT=sel[:, gi * 128:(gi + 1) * 128],
                                     rhs=src[:], start=True, stop=True)
                    bc = bc_ps[:]
                z0 = z0_tiles.pop(g)
                xtn = xtn_tiles.pop(g)
                zb = ofp.tile([P, GROUP], dt.bfloat16, tag="zb")
                nc.vector.tensor_tensor(out=zb[:], in0=z0[:], in1=bc,
                                        op=ALU.mult)
                zc = ofp.tile([P, GROUP], dt.bfloat16, tag="zc")
                nc.vector.tensor_scalar(out=zc[:], in0=zb[:],
                                        scalar1=gam[:, 0:1],
                                        scalar2=bet[:, 0:1],
                                        op0=ALU.mult, op1=ALU.add)
                of = ofp.tile([P, GROUP], dt.bfloat16, tag="of")
                nc.vector.tensor_tensor(out=of[:], in0=zc[:], in1=xtn[:],
                                        op=ALU.add)
                nc.sync.dma_start(out=OUT[:, nsl], in_=of[:])

            # --- emission: 3-group scatter prefetch; PE order per iteration
            # is L1(g) j0-j2, scat(g+3), L1(g) j3, L2(g), stats(g-1) so no PE
            # instruction waits on same-iteration DVE/ACT work (the j3 PSUM
            # bank is freed by silu(g,0) during the scatter burst). P3 drains
            # are paced evenly across iterations. ---
            g_of = []
            for bi, block in enumerate(blocks):
                for gi, g in enumerate(block):
                    g_of.append((g, gi, bi, len(block)))
            n = len(g_of)

            p3_queue = []
            stats_pending = None
            for j in range(min(3, n)):
                scat(g_of[j][0])
            for idx, (g, gi, bi, bsz) in enumerate(g_of):
                l1a(g)
                if idx + 3 < n:
                    scat(g_of[idx + 3][0])
                l1b(g)
                l2z(g)
                if stats_pending is not None:
                    pg, pgi, pbi, pbsz = stats_pending
                    stats(pg, pgi, pbi, pbsz)
                    if pgi == pbsz - 1:
                        bcinfo = phase2(pbi, pbsz)
                        p3_queue.extend((gg, ggi, bcinfo)
                                        for ggi, gg in enumerate(blocks[pbi]))
                stats_pending = (g, gi, bi, bsz)
                remaining = n - idx - 1
                k = (len(p3_queue) + max(remaining, 1) - 1) // max(remaining, 1)
                for _ in range(min(k, len(p3_queue))):
                    phase3(*p3_queue.pop(0))
            pg, pgi, pbi, pbsz = stats_pending
            stats(pg, pgi, pbi, pbsz)
            bcinfo = phase2(pbi, pbsz)
            p3_queue.extend((gg, ggi, bcinfo)
                            for ggi, gg in enumerate(blocks[pbi]))
            while p3_queue:
                phase3(*p3_queue.pop(0))

    nc.compile()
    return nc


# --------------------------------------------------------------------------
# host-side sharding / packing
# --------------------------------------------------------------------------

def _preprocess(inputs, n_cores, nodes_per_core):
    nf = np.ascontiguousarray(np.asarray(inputs["node_features"], np.float32))
    ef = np.ascontiguousarray(np.asarray(inputs["edge_features"], np.float32))
    src = np.asarray(inputs["src_indices"]).astype(np.int64)
    W1 = np.asarray(inputs["W1"], np.float32)
    b1 = np.asarray(inputs["b1"], np.float32)
    W2 = np.asarray(inputs["W2"], np.float32)
    b2 = np.asarray(inputs["b2"], np.float32)
    gam = np.asarray(inputs["ln_gamma"], np.float32)
    bet = np.asarray(inputs["ln_beta"], np.float32)

    n_nodes, d = nf.shape
    n_edges = ef.shape[0]
    tiles_per_core = nodes_per_core // P
    n_tiles = n_cores * tiles_per_core
    n_groups = nodes_per_core // GROUP
    sizes = _block_sizes(n_groups)
    bmax = max(sizes)
    bmax_late = max(sizes[1:], default=0)

    # subtile = (tile, half); width-64 one-hot
    sub_e = src // 64                      # global subtile id, 2*n_tiles
    lid64 = (src % 64).astype(np.int64)
    scounts = np.bincount(sub_e, minlength=2 * n_tiles)
    scnt = np.ceil(scounts / P).astype(int).reshape(n_tiles, 2)

    # serpentine deal of tiles (desc by total chunk count) into cores, then
    # sort each core's tiles desc so the shared per-position budget is tight
    tot = scnt.sum(axis=1)
    order_t = np.argsort(-tot, kind="stable")
    assign = np.empty((n_cores, tiles_per_core), np.int64)
    for r in range(tiles_per_core):
        row = order_t[r * n_cores:(r + 1) * n_cores]
        if r % 2 == 1:
            row = row[::-1]
        assign[:, r] = row
    core_of_tile = np.empty(n_tiles, np.int64)
    pos_of_tile = np.empty(n_tiles, np.int64)
    for k in range(n_cores):
        core_of_tile[assign[k]] = k
        pos_of_tile[assign[k]] = np.arange(tiles_per_core)

    cisA = np.maximum(np.max(scnt[assign, 0], axis=0), 1).astype(int)
    cisB = np.maximum(np.max(scnt[assign, 1], axis=0), 1).astype(int)
    cist = cisA + cisB
    coff = np.concatenate([[0], np.cumsum(cist)]).astype(int)
    CH = int(coff[-1])

    # edge slot placement (within-subtile rank -> chunk, partition)
    order = np.argsort(src, kind="stable")
    snode = src[order]
    ssub = snode // 64
    sstarts = np.zeros(2 * n_tiles, np.int64)
    np.cumsum(scounts[:-1], out=sstarts[1:])
    rank = np.arange(n_edges, dtype=np.int64) - sstarts[ssub]
    chunk = rank // P
    part = rank % P
    stile = ssub // 2
    shalf = ssub % 2
    score = core_of_tile[stile]
    spos = pos_of_tile[stile]
    cslot = coff[spos] + np.where(shalf == 1, cisA[spos], 0) + chunk

    earr = np.zeros((n_cores, CH, P, d), BF16)
    earr[score, cslot, part] = ef[order].astype(BF16)
    oarr = np.zeros((n_cores, CH, P, 64), FP8)
    oarr[score, cslot, part, lid64[order]] = 1.0

    # pack pk bytes: per position, edge seg then one-hot seg
    segs = []
    for i in range(tiles_per_core):
        a, b = int(coff[i]), int(coff[i + 1])
        ci = b - a
        eseg = np.ascontiguousarray(
            earr[:, a:b].transpose(0, 2, 1, 3)).reshape(n_cores, P, ci * d)
        segs.append(eseg.view(np.uint8).reshape(n_cores, P, ci * 256))
        oseg = np.ascontiguousarray(
            oarr[:, a:b].transpose(0, 2, 1, 3)).reshape(n_cores, P, ci * 64)
        segs.append(oseg.view(np.uint8))
    PKa = np.ascontiguousarray(np.concatenate(segs, axis=2))

    # node features packed in assigned-tile order, transposed, bf16
    nfp = np.zeros((n_tiles * P, d), np.float32)
    nfp[:n_nodes] = nf
    tiles_nf = nfp.reshape(n_tiles, P, d)
    NTBa = np.empty((n_cores, P, nodes_per_core), BF16)
    for k in range(n_cores):
        blk = tiles_nf[assign[k]].reshape(nodes_per_core, d)
        NTBa[k] = blk.T.astype(BF16)

    # fold layernorm mean-centering into W2 / b2
    W2c = W2 - W2.mean(axis=1, keepdims=True)
    b2c = (b2 - b2.mean()).astype(np.float32)

    W1P = np.ascontiguousarray(
        W1.reshape(2, P, 4, P).transpose(1, 0, 2, 3).reshape(P, 1024)).astype(BF16)
    W2P = np.ascontiguousarray(
        W2c.reshape(4, P, P).transpose(1, 0, 2).reshape(P, 512)).astype(BF16)
    B1P = np.ascontiguousarray(b1.reshape(4, P).T)
    B2P = np.ascontiguousarray(b2c.reshape(P, 1))
    GAMP = np.ascontiguousarray(gam.reshape(P, 1))
    BETP = np.ascontiguousarray(bet.reshape(P, 1))
    ONB = np.zeros((P, bmax * 128), np.float32)
    for g in range(bmax):
        ONB[:, g * 128 + g] = 1.0 / P
    ONB = ONB.astype(BF16)
    SELW = max(bmax_late, 1)
    SELa = np.zeros((P, SELW * 128), np.float32)
    for g in range(SELW):
        SELa[g, g * 128:(g + 1) * 128] = 1.0
    SELa = SELa.astype(BF16)

    in_maps = []
    for k in range(n_cores):
        in_maps.append({
            "pk": PKa[k], "ntb": NTBa[k],
            "w1p": W1P, "w2p": W2P, "b1p": B1P, "b2p": B2P,
            "gam": GAMP, "bet": BETP, "onb": ONB, "sel": SELa,
        })
    cis = tuple((int(a), int(b)) for a, b in zip(cisA, cisB))
    return in_maps, cis, assign


def _assemble(results, n_nodes, n_cores, nodes_per_core, assign):
    tiles_per_core = nodes_per_core // P
    n_tiles = n_cores * tiles_per_core
    full = np.empty((n_tiles, P, D), np.float32)
    for k in range(n_cores):
        outk = np.asarray(results[k]["out"]).astype(np.float32)  # [P, npc]
        full[assign[k]] = outk.T.reshape(tiles_per_core, P, D)
    return np.ascontiguousarray(full.reshape(n_tiles * P, D)[:n_nodes])


# --------------------------------------------------------------------------
# public entry point
# --------------------------------------------------------------------------

_AXON_SO = "/opt/axon/libaxon_pjrt.so"


def _ensure_ntff_hook():
    """Provide antenv.axon_hooks + register the ctypes NTFF profile hook
    (the agent image's antenv lacks axon_hooks, so boot degraded silently)."""
    import sys
    import types
    import ctypes
    import contextlib
    import os

    try:
        from antenv.axon_hooks import get_axon_ntff_profile_hook  # noqa: F401
        return
    except ImportError:
        pass
    import antenv

    m = types.ModuleType("antenv.axon_hooks")
    m._hook = None

    def set_axon_ntff_profile_hook(h):
        m._hook = h

    def get_axon_ntff_profile_hook():
        return m._hook

    m.set_axon_ntff_profile_hook = set_axon_ntff_profile_hook
    m.get_axon_ntff_profile_hook = get_axon_ntff_profile_hook
    sys.modules["antenv.axon_hooks"] = m
    antenv.axon_hooks = m

    if not os.path.exists(_AXON_SO):
        return
    lib = ctypes.CDLL(_AXON_SO)
    if not hasattr(lib, "axon_start_nrt_profile"):
        return
    lib.axon_start_nrt_profile.argtypes = [ctypes.POINTER(ctypes.c_int64),
                                           ctypes.c_size_t]
    lib.axon_start_nrt_profile.restype = ctypes.c_int64
    lib.axon_stop_nrt_profile.argtypes = [ctypes.c_char_p]
    lib.axon_stop_nrt_profile.restype = ctypes.c_int64

    @contextlib.contextmanager
    def _hook(output_dir, device_ids):
        import jax

        jax.devices()
        if device_ids:
            ids = (ctypes.c_int64 * len(device_ids))(*device_ids)
            rc = lib.axon_start_nrt_profile(ids, len(device_ids))
        else:
            rc = lib.axon_start_nrt_profile(None, 0)
        if rc != 0:
            raise RuntimeError(f"axon_start_nrt_profile rc={rc}")
        try:
            yield
        finally:
            n = lib.axon_stop_nrt_profile(str(output_dir).encode())
            if n < 0:
                raise RuntimeError(f"axon_stop_nrt_profile rc={n}")
            if n == 0:
                print("WARNING: NTFF capture wrote no files")

    m._hook = _hook


def _run(inputs, trace=False):
    if trace:
        _ensure_ntff_hook()
    n_nodes = np.asarray(inputs["node_features"]).shape[0]
    in_maps, cis, assign = _preprocess(inputs, N_CORES, NODES_PER_CORE)
    nc = _build(NODES_PER_CORE, cis, N_CORES)
    res = bass_utils.run_bass_kernel_spmd(
        nc, in_maps, core_ids=list(range(N_CORES)), trace=trace)
    out = _assemble(res.results, n_nodes, N_CORES, NODES_PER_CORE, assign)
    return out, res


def kernel(**inputs):
    out, _ = _run(inputs, trace=False)
    return out


def kernel_profiled(**inputs):
    out, res = _run(inputs, trace=True)
    return out, res
